# revision 10
# baseline (speedup 1.0000x reference)
"""Trainium2 Bass kernel for nn_Model_39676907886571 (per-head attention, S=2048, d=3).

Math (per head h, fully head-parallel, one head per NeuronCore):
  q_mat = query[h] @ x[h].T          (3, S)   -> q = q_mat viewed row-major as (S, 3)
  k_mat, v_mat likewise
  attn  = softmax(q @ k.T / sqrt(3)) (S, S)
  out   = (attn @ v).T               (3, S)

Device strategy (all on-chip, attention matrix never touches HBM):
  * Everything is computed in a "u-order" permutation of the sequence axis
    (u = 128*c + p  <->  t_true = 16*p + c), which is what 16 PE transposes of a
    contiguous (128, 48) tile naturally produce.  Softmax sums over the key axis
    are permutation-invariant; the query axis is un-permuted once at the end by
    a single strided DVE copy fused into the normalization multiply.
  * E^T = exp(K^T-chunks @ q^T / sqrt(3)) is computed with the key axis on
    partitions, so attn @ [1|v] needs no transposes and the softmax denominator
    falls out of the ones column.
"""

import numpy as np
from contextlib import ExitStack

import concourse.bass as bass
import concourse.tile as tile
from concourse import bacc, mybir
from concourse.masks import make_identity
from concourse import bass_utils

F32 = mybir.dt.float32
H, S, D = 8, 2048, 3
NC_CHUNKS = 16          # t-chunks of 128 (u-order blocks)
SQ = 512                # s-chunk width (one PSUM bank)
INV_SCALE = float(1.0 / np.sqrt(3.0))


def build_program(reps=1):
    nc = bacc.Bacc("TRN2", num_devices=H, debug=False)
    x_dram = nc.dram_tensor("x", (128, 48), F32, kind="ExternalInput")
    wt_dram = nc.dram_tensor("wt", (3, 9), F32, kind="ExternalInput")
    out_dram = nc.dram_tensor("out", (3, S), F32, kind="ExternalOutput")
    scratch = nc.dram_tensor("scratch", (3, 3 * S), F32, kind="Internal")

    with tile.TileContext(nc) as tc, ExitStack() as ctx:
        consts = ctx.enter_context(tc.tile_pool(name="consts", bufs=1))
        sb = ctx.enter_context(tc.tile_pool(name="sb", bufs=2 if reps > 1 else 1))
        es = ctx.enter_context(tc.tile_pool(name="es", bufs=3))
        ps = ctx.enter_context(tc.tile_pool(name="ps", bufs=2, space="PSUM"))

        ident = consts.tile([128, 128], F32)
        make_identity(nc, ident)
        ones4 = consts.tile([1, 4], F32)
        nc.vector.memset(ones4, 1.0)

        for _rep in range(reps):
            _build_body(nc, tc, sb, es, ps, ident, ones4, x_dram, wt_dram, out_dram, scratch)

    nc.compile()
    return nc


def _build_body(nc, tc, sb, es, ps, ident, ones4, x_dram, wt_dram, out_dram, scratch):
    if True:
        x_nat = sb.tile([128, 48], F32)
        nc.sync.dma_start(x_nat[:], x_dram.ap())
        wT_sb = sb.tile([3, 9], F32)
        nc.sync.dma_start(wT_sb[:], wt_dram.ap())

        # xT in u-order: xT_u[d, 128c+p] = x[16p+c, d]
        xT_ps = ps.tile([9, S], F32, tag="ps")
        for c in range(NC_CHUNKS):
            nc.tensor.transpose(
                xT_ps[0:3, 128 * c : 128 * (c + 1)], x_nat[:, 3 * c : 3 * (c + 1)], ident
            )
        xT_u = sb.tile([3, S], F32)
        nc.vector.tensor_copy(xT_u[:], xT_ps[0:3, :])

        # qkv = W9 @ xT  (9, S) in u-order
        qkv_ps = ps.tile([9, S], F32, tag="ps")
        for m in range(4):
            nc.tensor.matmul(
                qkv_ps[:, SQ * m : SQ * (m + 1)],
                lhsT=wT_sb[:],
                rhs=xT_u[:, SQ * m : SQ * (m + 1)],
                start=True,
                stop=True,
            )

        # un-permute u -> true order while copying PSUM -> SBUF:
        # qkv_sb[j, 16p+c] = qkv_ps[j, 128c+p]
        qkv_sb = sb.tile([9, S], F32)
        nc.vector.tensor_copy(
            qkv_sb.rearrange("j (p c) -> j p c", c=NC_CHUNKS),
            qkv_ps[0:9, :].rearrange("j (c p) -> j p c", p=128),
        )

        # natural layouts: nats[:, 48m + f] = flat_m[48*part + f]  (m = q,k,v)
        # natural (S, 3)-triple layout via a DRAM bounce (partition-crossing reshape)
        nats = sb.tile([128, 144], F32)
        scr = scratch.ap()
        for m in range(3):
            nc.sync.dma_start(scr[m, :], qkv_sb[3 * m : 3 * (m + 1), :])
        for m in range(3):
            nc.sync.dma_start(nats[:, 48 * m : 48 * (m + 1)], scr[m, :])

        # vplus quads: [1, v0, v1, v2] per chunk c -> lhsT for the attn@v matmuls
        vplus = sb.tile([128, 64], F32)
        nc.vector.memset(vplus[:], 1.0)
        nc.vector.tensor_copy(
            vplus.rearrange("p (c q) -> p c q", q=4)[:, :, 1:4],
            nats[:, 96:144].rearrange("p (c d) -> p c d", d=3),
        )

        # qT_u / kT_u (3, S) via PE transposes of natural chunks
        qT_u = sb.tile([3, S], F32)
        kT_u = sb.tile([3, S], F32)
        for src_off, dst in ((0, qT_u), (48, kT_u)):
            t_ps = ps.tile([9, S], F32, tag="ps")
            for c in range(NC_CHUNKS):
                nc.tensor.transpose(
                    t_ps[0:3, 128 * c : 128 * (c + 1)],
                    nats[:, src_off + 3 * c : src_off + 3 * (c + 1)],
                    ident,
                )
            nc.vector.tensor_copy(dst[:], t_ps[0:3, :])

        # ---------------- main attention loop ----------------
        # acc rows: [denom, o0, o1, o2], cols in u-order of s
        acc = sb.tile([4, S], F32)
        nc.vector.memset(acc[:], 0.0)

        for j in range(4):          # s-chunk (512 wide)
            for r in range(4):      # group of 4 t-chunks
                ps_t = ps.tile([128, S], F32, tag="ps")
                for i in range(4):
                    c = 4 * r + i
                    nc.tensor.matmul(
                        ps_t[:, SQ * i : SQ * (i + 1)],
                        lhsT=kT_u[:, 128 * c : 128 * (c + 1)],
                        rhs=qT_u[:, SQ * j : SQ * (j + 1)],
                        start=True,
                        stop=True,
                    )
                e_t = es.tile([128, S], F32)
                nc.scalar.activation(
                    e_t[:], ps_t[:], mybir.ActivationFunctionType.Exp, scale=INV_SCALE
                )
                for i in range(4):
                    c = 4 * r + i
                    nc.tensor.matmul(
                        ps_t[0:4, 0:SQ],
                        lhsT=vplus[:, 4 * c : 4 * (c + 1)],
                        rhs=e_t[:, SQ * i : SQ * (i + 1)],
                        start=(i == 0),
                        stop=(i == 3),
                    )
                nc.vector.tensor_add(
                    acc[:, SQ * j : SQ * (j + 1)],
                    acc[:, SQ * j : SQ * (j + 1)],
                    ps_t[0:4, 0:SQ],
                )

        # ---------------- epilogue ----------------
        recip = sb.tile([1, S], F32)
        nc.vector.reciprocal(recip[:], acc[0:1, :])
        bc_ps = ps.tile([9, S], F32, tag="ps")
        for m in range(4):
            nc.tensor.matmul(
                bc_ps[0:4, SQ * m : SQ * (m + 1)],
                lhsT=ones4[:],
                rhs=recip[:, SQ * m : SQ * (m + 1)],
                start=True,
                stop=True,
            )
        outv = sb.tile([4, S], F32)
        # fused un-permute of the s axis: outv[p, 16pp+c] = acc[p] * recip, cols u->true
        # (row 0 divides denom by itself -> 1.0, discarded; keeps partition base 0)
        nc.vector.tensor_mul(
            outv.rearrange("p (pp c) -> p pp c", c=NC_CHUNKS),
            acc.rearrange("p (c pp) -> p pp c", pp=128),
            bc_ps[0:4, :].rearrange("p (c pp) -> p pp c", pp=128),
        )
        nc.sync.dma_start(out_dram.ap(), outv[1:4, :])


_NC_CACHE = None


def _get_program():
    global _NC_CACHE
    if _NC_CACHE is None:
        _NC_CACHE = build_program()
    return _NC_CACHE


def kernel(x1, query, key_w, value, dropout_p=0):
    x1 = np.ascontiguousarray(np.asarray(x1, dtype=np.float32))
    query = np.asarray(query, dtype=np.float32)
    key_w = np.asarray(key_w, dtype=np.float32)
    value = np.asarray(value, dtype=np.float32)

    nc = _get_program()
    in_maps = []
    for h in range(H):
        w9t = np.ascontiguousarray(
            np.concatenate([query[h], key_w[h], value[h]], axis=0).T
        )  # (3, 9)
        in_maps.append({"x": x1[h].reshape(128, 48), "wt": w9t})

    res = bass_utils.run_bass_kernel_spmd(nc, in_maps, core_ids=list(range(H)))
    return np.stack([res.results[h]["out"] for h in range(H)])


# revision 46
# speedup vs baseline: 260.8892x; 260.8892x over previous
"""Trainium2 Bass kernel for nn_Model_39676907886571 (per-head attention, S=2048, d=3).

Math (per head h, fully head/data parallel, one head per NeuronCore):
  q_mat = query[h] @ x[h].T          (3, S)   -> q = q_mat viewed row-major as (S, 3)
  k_mat, v_mat likewise (the reshape is a memory-reinterpreting view, not a transpose)
  attn  = softmax(q @ k.T / sqrt(3)) (S, S)
  out   = (attn @ v).T               (3, S)

Device strategy (all on-chip, the S x S attention matrix never touches HBM):
  * qkv = W9 @ xT on the PE; a DRAM bounce reshapes the row-major flats into the
    "natural" (S, 3) triple layout that the weird view demands.
  * q^T / k^T are rebuilt by 16 PE transposes each, which produce a "u-order"
    permutation of the sequence axis (u = 128*c + p  <->  t_true = 16*p + c).
    Softmax sums over the key axis are permutation-invariant; the query axis is
    un-permuted at the end by a strided DVE write fused into the normalization.
  * E^T = exp(k-chunks^T @ q^T / sqrt(3)) keeps the key axis on partitions, so
    attn @ [1|v] needs no transposes and the softmax denominator falls out of
    the ones column of the [1|v] stationary operand.
  * Matmul operands are float32r (single-pass fp32 PE mode, 4x faster than the
    fp32hi/lo pair); PSUM ping-pong is managed manually so consecutive rounds
    only serialize through the exp (ACT is the bottleneck engine).
"""

import numpy as np
from contextlib import ExitStack

import concourse.bass as bass
import concourse.tile as tile
from concourse import bacc, mybir
from concourse import bass_utils

F32 = mybir.dt.float32
F32R = mybir.dt.float32r

H, S, D = 8, 2048, 3
NCH = 16                # t-chunks of 128 (u-order blocks)
SQ = 512                # s-chunk width (one PSUM bank)
INV_SCALE = float(1.0 / np.sqrt(3.0))


def _r(ap):
    """Bitcast an fp32 AP to float32r (same bits)."""
    return ap.bitcast(F32R)


def build_program(reps=1):
    nc = bacc.Bacc("TRN2", num_devices=H, debug=False)
    xt_dram = nc.dram_tensor("xt", (3, S), F32, kind="ExternalInput")
    wt_dram = nc.dram_tensor("wt", (3, 9), F32, kind="ExternalInput")
    out_dram = nc.dram_tensor("out", (3, S), F32, kind="ExternalOutput")
    scratch = nc.dram_tensor("scratch", (3, 3 * S), F32, kind="Internal")

    with tile.TileContext(nc) as tc, ExitStack() as ctx:
        consts = ctx.enter_context(tc.tile_pool(name="consts", bufs=1))
        sb = ctx.enter_context(tc.tile_pool(name="sb", bufs=2 if reps > 1 else 1))
        es = ctx.enter_context(tc.tile_pool(name="es", bufs=3))
        ping = ctx.enter_context(tc.tile_pool(name="ping", bufs=1, space="PSUM"))
        accp = ctx.enter_context(tc.tile_pool(name="accp", bufs=1, space="PSUM"))

        # constants (shared across reps)
        ident_f = consts.tile([128, 128], F32)
        from concourse.masks import make_identity

        make_identity(nc, ident_f)
        ident = consts.tile([128, 128], F32R)
        nc.vector.tensor_copy(ident[:], ident_f[:])
        onesq_f = consts.tile([128, 16], F32)
        nc.vector.memset(onesq_f, 1.0)
        onesq = consts.tile([128, 16], F32R)
        nc.vector.tensor_copy(onesq[:], onesq_f[:])
        ones4 = consts.tile([1, 4], F32R)
        nc.vector.tensor_copy(ones4[:], onesq_f[0:1, 0:4])
        # prewarm the ACT exp table so the ~2.7us table load overlaps the prologue
        warm = consts.tile([1, 1], F32)
        nc.scalar.activation(warm[:], onesq_f[0:1, 0:1], mybir.ActivationFunctionType.Exp)

        for _rep in range(reps):
            _build_body(nc, tc, sb, es, ping, accp, ident, onesq, ones4,
                        xt_dram, wt_dram, out_dram, scratch)

    nc.compile()
    return nc


def _build_body(nc, tc, sb, es, ping, accp, ident, onesq, ones4,
                xt_dram, wt_dram, out_dram, scratch):
    psA = ping.tile([128, 3 * SQ], F32, tag="A")
    psB = ping.tile([128, 3 * SQ], F32, tag="B")
    pst_of = lambda g: psA if g % 2 == 0 else psB

    wT_sb = sb.tile([3, 9], F32R)
    nc.scalar.dma_start(wT_sb[:], _r(wt_dram.ap()))
    xT = sb.tile([3, S], F32R)
    nc.sync.dma_start(xT[:, 0 : 2 * SQ], _r(xt_dram.ap()[:, 0 : 2 * SQ]))
    nc.scalar.dma_start(xT[:, 2 * SQ : S], _r(xt_dram.ap()[:, 2 * SQ : S]))

    # qkv = W9 @ xT  (9, S), true t-order; PSUM -> SBUF -> DRAM bounce -> nats
    for m in range(4):
        tgt = psA[0:9, SQ * m : SQ * (m + 1)] if m < 3 else psB[0:9, 0:SQ]
        nc.tensor.matmul(
            tgt,
            lhsT=wT_sb[:],
            rhs=xT[:, SQ * m : SQ * (m + 1)],
            start=True,
            stop=True,
        )
    qkv_sb = sb.tile([9, S], F32)
    nc.scalar.copy(qkv_sb[:, 0 : 3 * SQ], psA[0:9, :])
    nc.vector.tensor_copy(qkv_sb[:, 3 * SQ : S], psB[0:9, 0:SQ])

    # warm the PE pstate during the otherwise idle DMA-bounce window so the
    # first transposes/matmuls run at full clock (writes are dead; mm1(0)
    # overwrites the same PSUM region later)
    for _w in range(10):
        nc.tensor.transpose(_r(psB[0:128, SQ : SQ + 128]), ident[:], ident[:])

    # natural (S, 3)-triple layout via a DRAM bounce (partition-crossing
    # reshape); per-tensor stores/loads pipelined across the two HWDGE queues
    nats = sb.tile([128, 144], F32R)
    scr = scratch.ap()
    nc.sync.dma_start(scr[0, :], qkv_sb[0:3, :])
    nc.scalar.dma_start(scr[1, :], qkv_sb[3:6, :])
    nc.sync.dma_start(scr[2, :], qkv_sb[6:9, :])
    nc.scalar.dma_start(nats[:, 0:48], _r(scr[0, :]))
    nc.sync.dma_start(nats[:, 48:96], _r(scr[1, :]))
    nc.scalar.dma_start(nats[:, 96:144], _r(scr[2, :]))

    # vplus quads [1, v0, v1, v2] per chunk; built on the (otherwise idle) GPSIMD
    vplus = sb.tile([128, 64], F32R)
    nc.gpsimd.tensor_copy(vplus.rearrange("p (c q) -> p c q", q=4)[:, :, 0:1], onesq[:].unsqueeze(-1))
    for g in range(4):
        nc.gpsimd.tensor_copy(
            vplus.rearrange("p (c q) -> p c q", q=4)[:, 4 * g : 4 * (g + 1), 1:4],
            nats[:, 96 + 12 * g : 96 + 12 * (g + 1)].rearrange("p (c d) -> p c d", d=3),
        )

    # q^T / k^T in u-order via PE transposes of natural chunks.  Only the chunks
    # needed by round 0 are produced up front; the rest are interleaved into the
    # main loop's idle PE slots (writing to spare bank regions of the round's
    # PSUM tile after the exp has read it).
    qT_u = sb.tile([3, S], F32R)
    kT_u = sb.tile([3, S], F32R)

    def transpose_group_mm(src_off, grp, ps_region):
        for ci in range(4):
            c = 4 * grp + ci
            nc.tensor.transpose(
                _r(ps_region[0:3, 128 * ci : 128 * (ci + 1)]),
                nats[:, src_off + 3 * c : src_off + 3 * (c + 1)],
                ident[:],
            )

    def transpose_group_copy(dst, grp, ps_region):
        nc.vector.tensor_copy(dst[:, SQ * grp : SQ * (grp + 1)], ps_region[0:3, :])

    def transpose_group(dst, src_off, grp, ps_region):
        transpose_group_mm(src_off, grp, ps_region)
        transpose_group_copy(dst, grp, ps_region)

    # ---------------- main attention loop (software-pipelined) ----------------
    # Rounds of <=3 t-chunks (the PSUM tiles are 3 banks); the attn@[1|v]
    # accumulation lives in its own 1-bank PSUM accumulator per s-chunk, so the
    # only cross-round serialization is mm1(next) -> exp: ACT runs back-to-back.
    # The first two rounds are 2 chunks wide: their exp leaves PSUM bank 2 free,
    # which hosts in-loop transposes without any wait on the exp.
    # acc rows: [denom, o0, o1, o2], cols in u-order of s.
    ROUND_CHUNKS = [(0, 1), (2, 3), (4, 5, 6), (7, 8, 9), (10, 11, 12), (13, 14, 15)]
    NR = len(ROUND_CHUNKS)
    recip = sb.tile([1, S], F32R)
    bc_sb = sb.tile([4, S], F32R)
    outv = sb.tile([4, S], F32)
    accs = [accp.tile([4, SQ], F32, tag=f"acc{j % 2}", name=f"acc_j{j}") for j in range(4)]

    def mm1(g):
        j, r = divmod(g, NR)
        pst = pst_of(g)
        for i, c in enumerate(ROUND_CHUNKS[r]):
            nc.tensor.matmul(
                pst[:, SQ * i : SQ * (i + 1)],
                lhsT=kT_u[:, 128 * c : 128 * (c + 1)],
                rhs=qT_u[:, SQ * j : SQ * (j + 1)],
                start=True,
                stop=True,
            )

    # remaining transpose groups ride the idle PE slots: PE work at round g
    # (into the free bank 2 on the 2-wide rounds, else into bank 0 after the
    # exp's read); the PSUM->SBUF copy early in round g+1 (it overlaps that
    # round's exp); the consuming mm1 issues one or more rounds later.
    late_groups = {0: (kT_u, 48, 3), 1: (qT_u, 0, 1),
                   6: (qT_u, 0, 2), 7: (qT_u, 0, 3)}

    def late_region(g):
        pst = pst_of(g)
        return pst[:, 2 * SQ : 3 * SQ] if len(ROUND_CHUNKS[g % NR]) == 2 else pst[:, 0:SQ]

    def epilogue(j, bc_ps=None):
        # ---- per-s-chunk normalization, off the ACT critical path ----
        with nc.allow_low_precision(reason="float32r is 4-byte"):
            nc.vector.reciprocal(recip[:, SQ * j : SQ * (j + 1)], _r(accs[j][0:1, :]))
        if bc_ps is None:
            # mid-loop: broadcast on the idle GPSIMD
            bc = bc_sb[0:4, SQ * j : SQ * (j + 1)]
            nc.gpsimd.partition_broadcast(bc, recip[:, SQ * j : SQ * (j + 1)])
        else:
            # final chunk: PE is idle by now and its broadcast matmul is faster
            bc = bc_ps[0:4, :]
            nc.tensor.matmul(
                bc, lhsT=ones4[:], rhs=recip[:, SQ * j : SQ * (j + 1)],
                start=True, stop=True,
            )
        # normalization multiply fused with the u -> true-order un-permute of s:
        # outv[p, 16*pp + (4j+cc)] = acc[p, 128*cc + pp] * recip[...]
        nc.vector.tensor_mul(
            outv.rearrange("p (pp c) -> p pp c", c=NCH)[:, :, 4 * j : 4 * (j + 1)],
            accs[j][0:4, :].rearrange("p (c pp) -> p pp c", pp=128),
            bc.rearrange("p (c pp) -> p pp c", pp=128),
        )

    # prologue transpose groups: q0/k0 gate round 0; k1/k2 run behind mm1(0)
    # on the in-order PE (they execute during the first exps)
    transpose_group(qT_u, 0, 0, psA[:, 0:SQ])
    transpose_group(kT_u, 48, 0, psB[:, 0:SQ])
    mm1(0)
    transpose_group(kT_u, 48, 1, psB[:, SQ : 2 * SQ])
    transpose_group(kT_u, 48, 2, psB[:, 2 * SQ : 3 * SQ])

    for g in range(4 * NR):
        j, r = divmod(g, NR)
        pst = pst_of(g)
        width = SQ * len(ROUND_CHUNKS[r])
        e_t = es.tile([128, 3 * SQ], F32R)
        nc.scalar.activation(
            e_t[:, 0:width], pst[:, 0:width],
            mybir.ActivationFunctionType.Exp, scale=INV_SCALE,
        )
        if g - 1 in late_groups:
            dst, off, grp = late_groups[g - 1]
            transpose_group_copy(dst, grp, late_region(g - 1))
        if r == 0 and j >= 1:
            epilogue(j - 1)
        # next round's qk matmuls are independent of this exp: issue them first
        # so the in-order PE stream overlaps the exp (keeps ACT back-to-back)
        if g + 1 < 4 * NR:
            mm1(g + 1)
        for i, c in enumerate(ROUND_CHUNKS[r]):
            nc.tensor.matmul(
                accs[j][0:4, :],
                lhsT=vplus[:, 4 * c : 4 * (c + 1)],
                rhs=e_t[:, SQ * i : SQ * (i + 1)],
                start=(r == 0 and i == 0),
                stop=(r == NR - 1 and i == len(ROUND_CHUNKS[r]) - 1),
            )
        if g in late_groups:
            dst, off, grp = late_groups[g]
            transpose_group_mm(off, grp, late_region(g))

    epilogue(3)
    nc.sync.dma_start(out_dram.ap(), outv[1:4, :])


_NC_CACHE = None


def _get_program():
    global _NC_CACHE
    if _NC_CACHE is None:
        _NC_CACHE = build_program()
    return _NC_CACHE


def kernel(x1, query, key_w, value, dropout_p=0):
    x1 = np.asarray(x1, dtype=np.float32)
    query = np.asarray(query, dtype=np.float32)
    key_w = np.asarray(key_w, dtype=np.float32)
    value = np.asarray(value, dtype=np.float32)

    nc = _get_program()
    in_maps = []
    for h in range(H):
        w9t = np.ascontiguousarray(
            np.concatenate([query[h], key_w[h], value[h]], axis=0).T
        )  # (3, 9)
        in_maps.append({"xt": np.ascontiguousarray(x1[h].T), "wt": w9t})

    res = bass_utils.run_bass_kernel_spmd(nc, in_maps, core_ids=list(range(H)))
    return np.stack([res.results[h]["out"] for h in range(H)])
